# revision 16
# baseline (speedup 1.0000x reference)
"""DiffLlama attention (B=1, S=2048, HID=2048, H=16, KVH=8, D=128) on 8 TRN2 cores.

Sharding: tensor-parallel over the 8 "effective" (differential) heads.
Core i owns query heads (i, i+8), kv heads (i//2, 4+i//2), and the matching
256-column slice of the v_cat / output projection. o_proj is row-sharded;
partial products are summed on the host. attn_weights upper triangle is never
written on device (PJRT zero-fills outputs), matching softmax's exact zeros.

All matmuls run in float32r (full-rate PE mode, ~2^-12 effective rounding).
"""
import math
import os
import tempfile
import numpy as np
from contextlib import ExitStack

import concourse.bass as bass
import concourse.tile as tile
from concourse import bacc, mybir
from concourse.bass_utils import run_bass_kernel_spmd

B, S, HID = 1, 2048, 2048
H, KVH, D = 16, 8, 128
NCORES = 8
LAYER_IDX = 1
LAMBDA_INIT = 0.8 - 0.6 * float(np.exp(-0.3 * LAYER_IDX))
EPS = 1e-6
INV_SQRT_D = 1.0 / math.sqrt(D)

F32 = mybir.dt.float32
F32R = mybir.dt.float32r
AF = mybir.ActivationFunctionType

NQT = S // 128   # 16 query/key row tiles
NCH = S // 512   # 4 seq chunks

TRACE = False          # set by test.py to profile
DEBUG = False          # extra intermediate outputs for debugging
LAST_EXEC_NS = None
LAST_RESULTS = None
_CACHED_NC = None


def _build():
    nc = bacc.Bacc("TRN2", target_bir_lowering=False, debug=False)

    hsT = nc.dram_tensor("hsT", [HID, S], F32R, kind="ExternalInput")
    wq = nc.dram_tensor("wq", [HID, 256], F32R, kind="ExternalInput")
    wk = nc.dram_tensor("wk", [HID, 256], F32R, kind="ExternalInput")
    wv = nc.dram_tensor("wv", [HID, 256], F32R, kind="ExternalInput")
    wo = nc.dram_tensor("wo", [256, HID], F32R, kind="ExternalInput")
    cosq = nc.dram_tensor("cosq", [D, S], F32, kind="ExternalInput")
    sinq = nc.dram_tensor("sinq", [D, S], F32, kind="ExternalInput")
    cosk = nc.dram_tensor("cosk", [D, S], F32, kind="ExternalInput")
    sink = nc.dram_tensor("sink", [D, S], F32, kind="ExternalInput")
    lamv = nc.dram_tensor("lamv", [128, 1], F32, kind="ExternalInput")
    maskq = nc.dram_tensor("maskq", [128, 128], F32, kind="ExternalInput")
    maskt = nc.dram_tensor("maskt", [128, 128], F32, kind="ExternalInput")
    idm = nc.dram_tensor("idm", [128, 128], F32, kind="ExternalInput")

    attnw = nc.dram_tensor("attnw", [2, S, S], F32, kind="ExternalOutput")
    outp = nc.dram_tensor("outp", [S, HID], F32, kind="ExternalOutput")
    if DEBUG:
        dbg_et = nc.dram_tensor("dbg_et", [128, 512], F32R, kind="ExternalOutput")
        dbg_ao = nc.dram_tensor("dbg_ao", [2, 128, 256], F32, kind="ExternalOutput")
        dbg_act = nc.dram_tensor("dbg_act", [128, 256], F32, kind="ExternalOutput")
        dbg_atT = nc.dram_tensor("dbg_atT", [128, 256], F32R, kind="ExternalOutput")
        dbg_v = nc.dram_tensor("dbg_v", [128, 256], F32R, kind="ExternalOutput")

    with tile.TileContext(nc) as tc, ExitStack() as ctx:
        # ---- persistent pools ----
        psum = ctx.enter_context(tc.tile_pool(name="psum", bufs=8, space="PSUM"))
        qk = ctx.enter_context(tc.tile_pool(name="qk", bufs=1))
        vp = ctx.enter_context(tc.tile_pool(name="vp", bufs=16))
        wop = ctx.enter_context(tc.tile_pool(name="wop", bufs=2))
        consts = ctx.enter_context(tc.tile_pool(name="consts", bufs=1))

        qT = [qk.tile([128, S], F32R, tag=f"qT{h}", name=f"qT{h}") for h in range(2)]
        kT = [qk.tile([128, S], F32R, tag=f"kT{h}", name=f"kT{h}") for h in range(2)]
        v_sb = [vp.tile([128, 256], F32R, tag="v", name=f"v{j}") for j in range(NQT)]
        wo_sb = [wop.tile([128, S], F32R, tag="wo", name=f"wo{j}") for j in range(2)]

        maskq_sb = consts.tile([128, 128], F32, tag="mq")
        maskt_sb = consts.tile([128, 128], F32, tag="mt")
        idm_sb = consts.tile([128, 128], F32, tag="idm")
        lam_sb = consts.tile([128, 1], F32, tag="lam")
        eps_sb = consts.tile([128, 1], F32, tag="eps")
        nc.vector.memset(eps_sb[:], EPS)
        nc.sync.dma_start(maskq_sb[:], maskq.ap())
        nc.sync.dma_start(maskt_sb[:], maskt.ap())
        nc.sync.dma_start(idm_sb[:], idm.ap())
        nc.sync.dma_start(lam_sb[:], lamv.ap())
        for t2 in range(2):
            nc.sync.dma_start(wo_sb[t2][:], wo.ap()[t2 * 128:(t2 + 1) * 128, :])

        # ================= stage 1: projections + RoPE =================
        with ExitStack() as s1:
            wts = s1.enter_context(tc.tile_pool(name="wts", bufs=1))
            hsp = s1.enter_context(tc.tile_pool(name="hsp", bufs=4))
            trig = s1.enter_context(tc.tile_pool(name="trig", bufs=1))
            rope = s1.enter_context(tc.tile_pool(name="rope", bufs=3))

            # resident weights: [128, 16, 256] (hid-tile-major)
            wq_sb = wts.tile([128, NQT, 256], F32R, tag="wq")
            wk_sb = wts.tile([128, NQT, 256], F32R, tag="wk")
            wv_sb = wts.tile([128, NQT, 256], F32R, tag="wv")
            for name, dsrc, dst in (("wq", wq, wq_sb), ("wk", wk, wk_sb), ("wv", wv, wv_sb)):
                nc.sync.dma_start(dst[:], dsrc.ap().rearrange("(t p) m -> p t m", p=128))

            cs = {}
            for name, dsrc in (("cosq", cosq), ("sinq", sinq), ("cosk", cosk), ("sink", sink)):
                t = trig.tile([128, S], F32, tag=name)
                nc.sync.dma_start(t[:], dsrc.ap())
                cs[name] = t

            for n in range(NCH):
                sl = slice(n * 512, (n + 1) * 512)
                ps_q = [psum.tile([128, 512], F32, tag="ps", name=f"psq{j}") for j in range(2)]
                ps_k = [psum.tile([128, 512], F32, tag="ps", name=f"psk{j}") for j in range(2)]
                # one bank per v m-tile: a second accumulation group in the same
                # bank would clear the first group's has_written bits on start
                ps_v = [psum.tile([128, 512], F32, tag="ps", name=f"psv{j}") for j in range(4)]
                for kt in range(NQT):
                    hst = hsp.tile([128, 512], F32R, tag="hs")
                    nc.sync.dma_start(hst[:], hsT.ap()[kt * 128:(kt + 1) * 128, sl])
                    st, sp = kt == 0, kt == NQT - 1
                    for h in range(2):
                        nc.tensor.matmul(ps_q[h][:], wq_sb[:, kt, h * 128:(h + 1) * 128],
                                         hst[:], start=st, stop=sp)
                        nc.tensor.matmul(ps_k[h][:], wk_sb[:, kt, h * 128:(h + 1) * 128],
                                         hst[:], start=st, stop=sp)
                    for m in range(4):
                        nc.tensor.matmul(ps_v[m][:, :256],
                                         hst[:, m * 128:(m + 1) * 128], wv_sb[:, kt, :],
                                         start=st, stop=sp)
                # v copies (rounding to f32r)
                for m in range(4):
                    nc.vector.tensor_copy(v_sb[n * 4 + m][:], ps_v[m][:, :256])
                if DEBUG and n == 0:
                    nc.sync.dma_start(dbg_v.ap(), v_sb[0][:])
                # RoPE on q (scaled tables) and k
                for (ps_x, dstT, cname, sname) in (
                    (ps_q[0], qT[0], "cosq", "sinq"), (ps_q[1], qT[1], "cosq", "sinq"),
                    (ps_k[0], kT[0], "cosk", "sink"), (ps_k[1], kT[1], "cosk", "sink"),
                ):
                    xs = rope.tile([128, 512], F32, tag="xs")
                    nc.vector.tensor_copy(xs[:], ps_x[:])
                    xsh = rope.tile([128, 512], F32, tag="xsh")
                    nc.sync.dma_start(xsh[0:64, :], xs[64:128, :])
                    nc.sync.dma_start(xsh[64:128, :], xs[0:64, :])
                    t1 = rope.tile([128, 512], F32, tag="t1")
                    nc.vector.tensor_mul(t1[:], xs[:], cs[cname][:, sl])
                    t2 = rope.tile([128, 512], F32, tag="t2")
                    nc.vector.tensor_mul(t2[:], xsh[:], cs[sname][:, sl])
                    nc.vector.tensor_add(dstT[:, sl], t1[:], t2[:])

        # ================= stage 2: attention =================
        with ExitStack() as s2:
            etp = s2.enter_context(tc.tile_pool(name="etp", bufs=16))
            ewp = s2.enter_context(tc.tile_pool(name="ewp", bufs=6))
            aop = s2.enter_context(tc.tile_pool(name="aop", bufs=8))
            dp = s2.enter_context(tc.tile_pool(name="dp", bufs=24))
            sp = s2.enter_context(tc.tile_pool(name="sp", bufs=3))
            osp = s2.enter_context(tc.tile_pool(name="osp", bufs=4))

            for Q in range(NCH):
                kmax = 4 * (Q + 1)
                qsl = slice(Q * 512, (Q + 1) * 512)
                AO = {}
                for h in range(2):
                    # (a) transposed score strips -> exp -> ET[kt]
                    ET = []
                    for kt in range(kmax):
                        ps = psum.tile([128, 512], F32, tag="ps")
                        nc.tensor.matmul(ps[:], kT[h][:, kt * 128:(kt + 1) * 128],
                                         qT[h][:, qsl], start=True, stop=True)
                        if kt >= 4 * Q:
                            sub = slice((kt - 4 * Q) * 128, (kt - 4 * Q) * 128 + 128)
                            nc.vector.tensor_add(ps[:, sub], ps[:, sub], maskt_sb[:])
                        et = etp.tile([128, 512], F32R, tag="et")
                        nc.scalar.activation(et[:], ps[:], AF.Exp)
                        if DEBUG and Q == 0 and h == 0 and kt == 0:
                            nc.sync.dma_start(dbg_et.ap(), et[:])
                        ET.append(et)
                    # (b) natural scores -> exp(+rowsum) -> normalize -> attnw out
                    RD = []
                    for qi in range(4):
                        qt = 4 * Q + qi
                        W = (qt + 1) * 128
                        nchq = (W + 511) // 512
                        dparts = []
                        ews = []
                        for c in range(nchq):
                            w = min(512, W - c * 512)
                            ps = psum.tile([128, 512], F32, tag="ps")
                            nc.tensor.matmul(ps[:, :w], qT[h][:, qt * 128:(qt + 1) * 128],
                                             kT[h][:, c * 512:c * 512 + w],
                                             start=True, stop=True)
                            if c == nchq - 1:
                                nc.vector.tensor_add(ps[:, w - 128:w], ps[:, w - 128:w],
                                                     maskq_sb[:])
                            ew = ewp.tile([128, 512], F32, tag="ew")
                            dpt = dp.tile([128, 1], F32, tag="dp")
                            nc.scalar.activation(ew[:, :w], ps[:, :w], AF.Exp,
                                                 accum_out=dpt[:])
                            dparts.append(dpt)
                            ews.append((ew, w, c))
                        den = dparts[0]
                        for other in dparts[1:]:
                            den2 = dp.tile([128, 1], F32, tag="dp")
                            nc.vector.tensor_add(den2[:], den[:], other[:])
                            den = den2
                        rd = dp.tile([128, 1], F32, tag="rd")
                        nc.vector.reciprocal(rd[:], den[:])
                        RD.append(rd)
                        for (ew, w, c) in ews:
                            nc.vector.tensor_scalar_mul(ew[:, :w], ew[:, :w], rd[:])
                            nc.sync.dma_start(
                                attnw.ap()[h, qt * 128:(qt + 1) * 128, c * 512:c * 512 + w],
                                ew[:, :w])
                    # (c) attn_out, normalized into SBUF
                    for qi in range(4):
                        qt = 4 * Q + qi
                        po = psum.tile([128, 512], F32, tag="ps")
                        for kt in range(qt + 1):
                            nc.tensor.matmul(po[:, :256], ET[kt][:, qi * 128:(qi + 1) * 128],
                                             v_sb[kt][:], start=(kt == 0), stop=(kt == qt))
                        ao = aop.tile([128, 256], F32, tag="ao")
                        nc.vector.tensor_scalar_mul(ao[:], po[:, :256], RD[qi][:])
                        if DEBUG and Q == 0 and qi == 0:
                            nc.sync.dma_start(dbg_ao.ap()[h], ao[:])
                        AO[(h, qi)] = ao
                # (d/e) diff + RMSNorm + transpose + o_proj partial
                for qi in range(4):
                    qt = 4 * Q + qi
                    dift = sp.tile([128, 256], F32, tag="dift")
                    nc.vector.tensor_scalar_mul(dift[:], AO[(1, qi)][:], lam_sb[:])
                    dif = sp.tile([128, 256], F32, tag="dif")
                    nc.vector.tensor_sub(dif[:], AO[(0, qi)][:], dift[:])
                    sq = sp.tile([128, 256], F32, tag="sq")
                    ssum = dp.tile([128, 1], F32, tag="ss")
                    nc.scalar.activation(sq[:], dif[:], AF.Square, accum_out=ssum[:])
                    rms = dp.tile([128, 1], F32, tag="rms")
                    nc.scalar.activation(rms[:], ssum[:], AF.Sqrt, scale=1.0 / 256.0,
                                         bias=eps_sb[:])
                    rstd = dp.tile([128, 1], F32, tag="rstd")
                    nc.vector.reciprocal(rstd[:], rms[:])
                    act = sp.tile([128, 256], F32, tag="act")
                    nc.vector.tensor_scalar_mul(act[:], dif[:], rstd[:])
                    pt = psum.tile([128, 512], F32, tag="ps")
                    nc.tensor.transpose(pt[:, 0:128], act[:, 0:128], idm_sb[:])
                    nc.tensor.transpose(pt[:, 128:256], act[:, 128:256], idm_sb[:])
                    atT = sp.tile([128, 256], F32R, tag="atT")
                    nc.vector.tensor_copy(atT[:], pt[:, :256])
                    if DEBUG and Q == 0 and qi == 0:
                        nc.sync.dma_start(dbg_act.ap(), act[:])
                        nc.sync.dma_start(dbg_atT.ap(), atT[:])
                    for nb in range(4):
                        pso = psum.tile([128, 512], F32, tag="ps")
                        for k2 in range(2):
                            nc.tensor.matmul(pso[:], atT[:, k2 * 128:(k2 + 1) * 128],
                                             wo_sb[k2][:, nb * 512:(nb + 1) * 512],
                                             start=(k2 == 0), stop=(k2 == 1))
                        osb = osp.tile([128, 512], F32, tag="osb")
                        nc.scalar.copy(osb[:], pso[:])
                        nc.sync.dma_start(
                            outp.ap()[qt * 128:(qt + 1) * 128, nb * 512:(nb + 1) * 512],
                            osb[:])
    nc.compile()
    return nc


def _get_nc():
    global _CACHED_NC
    if _CACHED_NC is None:
        _CACHED_NC = _build()
    return _CACHED_NC


def kernel(hidden_states, cos, sin, Wq, Wk, Wv, Wo,
           lambda_q1, lambda_k1, lambda_q2, lambda_k2):
    global LAST_EXEC_NS
    f = np.float32
    hs = np.asarray(hidden_states, f)[0]          # [S, HID]
    hsT = np.ascontiguousarray(hs.T)              # [HID, S]
    cosT = np.ascontiguousarray(np.asarray(cos, f).T)   # [D, S]
    sinT = np.ascontiguousarray(np.asarray(sin, f).T)
    sgn = np.concatenate([-np.ones(64, f), np.ones(64, f)])[:, None]
    cosq = cosT * f(INV_SQRT_D)
    sinq = sinT * sgn * f(INV_SQRT_D)
    cosk = cosT
    sink = sinT * sgn

    lam1 = np.exp(np.sum(np.asarray(lambda_q1, f) * np.asarray(lambda_k1, f),
                         dtype=np.float64))
    lam2 = np.exp(np.sum(np.asarray(lambda_q2, f) * np.asarray(lambda_k2, f),
                         dtype=np.float64))
    lam = f(lam1 - lam2 + LAMBDA_INIT)
    lamv = np.full((128, 1), lam, f)

    r = np.arange(128)
    maskq = np.where(r[None, :] <= r[:, None], f(0.0), f(-1e9)).astype(f)  # [sq, sk]
    maskt = np.ascontiguousarray(maskq.T)
    idm = np.eye(128, dtype=f)

    Wq_, Wk_, Wv_, Wo_ = (np.asarray(x, f) for x in (Wq, Wk, Wv, Wo))
    Wo_s = Wo_ * f(1.0 - LAMBDA_INIT)

    in_maps = []
    for i in range(NCORES):
        a, b = i // 2, 4 + i // 2
        wq_i = np.ascontiguousarray(
            np.concatenate([Wq_[:, i * D:(i + 1) * D],
                            Wq_[:, (i + 8) * D:(i + 9) * D]], axis=1))
        wk_i = np.ascontiguousarray(
            np.concatenate([Wk_[:, a * D:(a + 1) * D],
                            Wk_[:, b * D:(b + 1) * D]], axis=1))
        wv_i = np.ascontiguousarray(
            np.concatenate([Wv_[:, a * D:(a + 1) * D],
                            Wv_[:, b * D:(b + 1) * D]], axis=1))
        wo_i = np.ascontiguousarray(Wo_s[i * 256:(i + 1) * 256, :])
        in_maps.append(dict(hsT=hsT, wq=wq_i, wk=wk_i, wv=wv_i, wo=wo_i,
                            cosq=cosq, sinq=sinq, cosk=cosk, sink=sink,
                            lamv=lamv, maskq=maskq, maskt=maskt, idm=idm))

    nc = _get_nc()
    kw = {}
    if TRACE:
        import concourse.bass_utils as _bu
        _bu.upload_artifacts = lambda tmpdir: tmpdir  # no artifact store here
        os.makedirs("/root/problem/traces", exist_ok=True)
        tdir = tempfile.mkdtemp(prefix="trace_", dir="/root/problem/traces")
        kw = dict(trace=True, trace_cores=list(range(NCORES)), tmpdir=tdir)
    res = run_bass_kernel_spmd(nc, in_maps, core_ids=list(range(NCORES)), **kw)
    LAST_EXEC_NS = res.exec_time_ns
    global LAST_RESULTS
    LAST_RESULTS = res.results

    attn_full = np.zeros((1, H, S, S), f)
    out = np.zeros((S, HID), f)
    for i in range(NCORES):
        attn_full[0, i] = res.results[i]["attnw"][0]
        attn_full[0, i + 8] = res.results[i]["attnw"][1]
        out += res.results[i]["outp"]
    return out.reshape(B, S, HID), attn_full


# revision 17
# speedup vs baseline: 1.0522x; 1.0522x over previous
"""DiffLlama attention (B=1, S=2048, HID=2048, H=16, KVH=8, D=128) on 8 TRN2 cores.

Sharding: tensor-parallel over the 8 "effective" (differential) heads.
Core i owns query heads (i, i+8), kv heads (i//2, 4+i//2), and the matching
256-column slice of the v_cat / output projection. o_proj is row-sharded;
partial products are summed on the host. attn_weights upper triangle is never
written on device (PJRT zero-fills outputs), matching softmax's exact zeros.

All matmuls run in float32r (full-rate PE mode, ~2^-12 effective rounding).
"""
import math
import os
import tempfile
import numpy as np
from contextlib import ExitStack

import concourse.bass as bass
import concourse.tile as tile
from concourse import bacc, mybir
from concourse.bass_utils import run_bass_kernel_spmd

B, S, HID = 1, 2048, 2048
H, KVH, D = 16, 8, 128
NCORES = 8
LAYER_IDX = 1
LAMBDA_INIT = 0.8 - 0.6 * float(np.exp(-0.3 * LAYER_IDX))
EPS = 1e-6
INV_SQRT_D = 1.0 / math.sqrt(D)

F32 = mybir.dt.float32
F32R = mybir.dt.float32r
AF = mybir.ActivationFunctionType

NQT = S // 128   # 16 query/key row tiles
NCH = S // 512   # 4 seq chunks

TRACE = False          # set by test.py to profile
DEBUG = False          # extra intermediate outputs for debugging
LAST_EXEC_NS = None
LAST_RESULTS = None
_CACHED_NC = None


def _build():
    nc = bacc.Bacc("TRN2", target_bir_lowering=False, debug=False)

    hsT = nc.dram_tensor("hsT", [HID, S], F32R, kind="ExternalInput")
    wq = nc.dram_tensor("wq", [HID, 256], F32R, kind="ExternalInput")
    wk = nc.dram_tensor("wk", [HID, 256], F32R, kind="ExternalInput")
    wv = nc.dram_tensor("wv", [HID, 256], F32R, kind="ExternalInput")
    wo = nc.dram_tensor("wo", [256, HID], F32R, kind="ExternalInput")
    cosq = nc.dram_tensor("cosq", [D, S], F32, kind="ExternalInput")
    sinq = nc.dram_tensor("sinq", [D, S], F32, kind="ExternalInput")
    cosk = nc.dram_tensor("cosk", [D, S], F32, kind="ExternalInput")
    sink = nc.dram_tensor("sink", [D, S], F32, kind="ExternalInput")
    lamv = nc.dram_tensor("lamv", [128, 1], F32, kind="ExternalInput")
    maskq = nc.dram_tensor("maskq", [128, 128], F32, kind="ExternalInput")
    maskt = nc.dram_tensor("maskt", [128, 128], F32, kind="ExternalInput")
    idm = nc.dram_tensor("idm", [128, 128], F32, kind="ExternalInput")

    attnw = nc.dram_tensor("attnw", [2, S, S], F32, kind="ExternalOutput")
    outp = nc.dram_tensor("outp", [S, HID], F32, kind="ExternalOutput")
    if DEBUG:
        dbg_et = nc.dram_tensor("dbg_et", [128, 512], F32R, kind="ExternalOutput")
        dbg_ao = nc.dram_tensor("dbg_ao", [2, 128, 256], F32, kind="ExternalOutput")
        dbg_act = nc.dram_tensor("dbg_act", [128, 256], F32, kind="ExternalOutput")
        dbg_atT = nc.dram_tensor("dbg_atT", [128, 256], F32R, kind="ExternalOutput")
        dbg_v = nc.dram_tensor("dbg_v", [128, 256], F32R, kind="ExternalOutput")

    with tile.TileContext(nc) as tc, ExitStack() as ctx:
        # ---- persistent pools ----
        psum = ctx.enter_context(tc.tile_pool(name="psum", bufs=4, space="PSUM"))
        qk = ctx.enter_context(tc.tile_pool(name="qk", bufs=1))
        vp = ctx.enter_context(tc.tile_pool(name="vp", bufs=16))
        wop = ctx.enter_context(tc.tile_pool(name="wop", bufs=2))
        consts = ctx.enter_context(tc.tile_pool(name="consts", bufs=1))

        qT = [qk.tile([128, S], F32R, tag=f"qT{h}", name=f"qT{h}") for h in range(2)]
        kT = [qk.tile([128, S], F32R, tag=f"kT{h}", name=f"kT{h}") for h in range(2)]
        v_sb = [vp.tile([128, 256], F32R, tag="v", name=f"v{j}") for j in range(NQT)]
        wo_sb = [wop.tile([128, S], F32R, tag="wo", name=f"wo{j}") for j in range(2)]

        maskq_sb = consts.tile([128, 128], F32, tag="mq")
        maskt_sb = consts.tile([128, 128], F32, tag="mt")
        idm_sb = consts.tile([128, 128], F32, tag="idm")
        lam_sb = consts.tile([128, 1], F32, tag="lam")
        eps_sb = consts.tile([128, 1], F32, tag="eps")
        nc.vector.memset(eps_sb[:], EPS)
        nc.gpsimd.dma_start(maskq_sb[:], maskq.ap())
        nc.gpsimd.dma_start(maskt_sb[:], maskt.ap())
        nc.gpsimd.dma_start(idm_sb[:], idm.ap())
        nc.gpsimd.dma_start(lam_sb[:], lamv.ap())
        for t2 in range(2):
            nc.gpsimd.dma_start(wo_sb[t2][:], wo.ap()[t2 * 128:(t2 + 1) * 128, :])

        # ================= stage 1: projections + RoPE =================
        with ExitStack() as s1:
            wts = s1.enter_context(tc.tile_pool(name="wts", bufs=1))
            hsp = s1.enter_context(tc.tile_pool(name="hsp", bufs=6))
            trig = s1.enter_context(tc.tile_pool(name="trig", bufs=1))
            rope = s1.enter_context(tc.tile_pool(name="rope", bufs=3))

            # resident weights: [128, 16, 256] (hid-tile-major)
            wq_sb = wts.tile([128, NQT, 256], F32R, tag="wq")
            wk_sb = wts.tile([128, NQT, 256], F32R, tag="wk")
            wv_sb = wts.tile([128, NQT, 256], F32R, tag="wv")
            for kt in range(NQT):
                for dsrc, dst in ((wq, wq_sb), (wk, wk_sb), (wv, wv_sb)):
                    nc.scalar.dma_start(dst[:, kt, :],
                                        dsrc.ap()[kt * 128:(kt + 1) * 128, :])

            cs = {}
            for name, dsrc in (("cosq", cosq), ("sinq", sinq), ("cosk", cosk), ("sink", sink)):
                t = trig.tile([128, S], F32, tag=name)
                for c4 in range(4):
                    nc.gpsimd.dma_start(t[:, c4 * 512:(c4 + 1) * 512],
                                        dsrc.ap()[:, c4 * 512:(c4 + 1) * 512])
                cs[name] = t

            for n in range(NCH):
                sl = slice(n * 512, (n + 1) * 512)
                ps4qk = psum.tile([128, 2048], F32, tag="ps4", name=f"ps4qk{n}", bufs=1)
                ps_q = [ps4qk[:, 0:512], ps4qk[:, 512:1024]]
                ps_k = [ps4qk[:, 1024:1536], ps4qk[:, 1536:2048]]
                # one bank per accumulation group (start= clears whole-bank bits)
                ps_v = [psum.tile([128, 512], F32, tag="ps", name=f"psv{j}") for j in range(4)]
                for kt in range(NQT):
                    hst = hsp.tile([128, 512], F32R, tag="hs")
                    nc.sync.dma_start(hst[:], hsT.ap()[kt * 128:(kt + 1) * 128, sl])
                    st, sp = kt == 0, kt == NQT - 1
                    for h in range(2):
                        nc.tensor.matmul(ps_q[h][:], wq_sb[:, kt, h * 128:(h + 1) * 128],
                                         hst[:], start=st, stop=sp)
                        nc.tensor.matmul(ps_k[h][:], wk_sb[:, kt, h * 128:(h + 1) * 128],
                                         hst[:], start=st, stop=sp)
                    for m in range(4):
                        nc.tensor.matmul(ps_v[m][:, :256],
                                         hst[:, m * 128:(m + 1) * 128], wv_sb[:, kt, :],
                                         start=st, stop=sp)
                # v copies (rounding to f32r)
                for m in range(4):
                    nc.vector.tensor_copy(v_sb[n * 4 + m][:], ps_v[m][:, :256])
                if DEBUG and n == 0:
                    nc.sync.dma_start(dbg_v.ap(), v_sb[0][:])
                # RoPE on q (scaled tables) and k
                for (ps_x, dstT, cname, sname) in (
                    (ps_q[0], qT[0], "cosq", "sinq"), (ps_q[1], qT[1], "cosq", "sinq"),
                    (ps_k[0], kT[0], "cosk", "sink"), (ps_k[1], kT[1], "cosk", "sink"),
                ):
                    xs = rope.tile([128, 512], F32, tag="xs")
                    nc.vector.tensor_copy(xs[:], ps_x[:])
                    xsh = rope.tile([128, 512], F32, tag="xsh")
                    nc.gpsimd.dma_start(xsh[0:64, :], xs[64:128, :])
                    nc.gpsimd.dma_start(xsh[64:128, :], xs[0:64, :])
                    t1 = rope.tile([128, 512], F32, tag="t1")
                    nc.vector.tensor_mul(t1[:], xs[:], cs[cname][:, sl])
                    t2 = rope.tile([128, 512], F32, tag="t2")
                    nc.vector.tensor_mul(t2[:], xsh[:], cs[sname][:, sl])
                    nc.vector.tensor_add(dstT[:, sl], t1[:], t2[:])

        # ================= stage 2: attention =================
        with ExitStack() as s2:
            etp = s2.enter_context(tc.tile_pool(name="etp", bufs=16))
            ewp = s2.enter_context(tc.tile_pool(name="ewp", bufs=3))
            aop = s2.enter_context(tc.tile_pool(name="aop", bufs=8))
            dp = s2.enter_context(tc.tile_pool(name="dp", bufs=24))
            sp = s2.enter_context(tc.tile_pool(name="sp", bufs=3))
            osp = s2.enter_context(tc.tile_pool(name="osp", bufs=3))

            for Q in range(NCH):
                kmax = 4 * (Q + 1)
                qsl = slice(Q * 512, (Q + 1) * 512)
                AO = {}
                for h in range(2):
                    # (a) transposed score strips -> exp -> ET[kt]
                    ET = []
                    for kt in range(kmax):
                        ps = psum.tile([128, 512], F32, tag="ps", name=f"pst{kt}")
                        nc.tensor.matmul(ps[:], kT[h][:, kt * 128:(kt + 1) * 128],
                                         qT[h][:, qsl], start=True, stop=True)
                        if kt >= 4 * Q:
                            sub = slice((kt - 4 * Q) * 128, (kt - 4 * Q) * 128 + 128)
                            nc.vector.tensor_add(ps[:, sub], ps[:, sub], maskt_sb[:])
                        et = etp.tile([128, 512], F32R, tag="et")
                        nc.scalar.activation(et[:], ps[:], AF.Exp)
                        if DEBUG and Q == 0 and h == 0 and kt == 0:
                            nc.sync.dma_start(dbg_et.ap(), et[:])
                        ET.append(et)
                    # (b) natural scores -> exp(+rowsum) -> normalize -> attnw out
                    RD = []
                    for qi in range(4):
                        qt = 4 * Q + qi
                        W = (qt + 1) * 128
                        nchq = (W + 511) // 512
                        ps4 = psum.tile([128, 2048], F32, tag="ps4", name=f"ps4n{qt}{h}", bufs=1)
                        for c in range(nchq):
                            w = min(512, W - c * 512)
                            nc.tensor.matmul(ps4[:, c * 512:c * 512 + w],
                                             qT[h][:, qt * 128:(qt + 1) * 128],
                                             kT[h][:, c * 512:c * 512 + w],
                                             start=True, stop=True)
                        nc.vector.tensor_add(ps4[:, W - 128:W], ps4[:, W - 128:W],
                                             maskq_sb[:])
                        ew = ewp.tile([128, 2048], F32, tag="ew")
                        den = dp.tile([128, 1], F32, tag="dp")
                        nc.scalar.activation(ew[:, :W], ps4[:, :W], AF.Exp,
                                             accum_out=den[:])
                        rd = dp.tile([128, 1], F32, tag="rd")
                        nc.vector.reciprocal(rd[:], den[:])
                        RD.append(rd)
                        nc.vector.tensor_scalar_mul(ew[:, :W], ew[:, :W], rd[:])
                        nc.sync.dma_start(
                            attnw.ap()[h, qt * 128:(qt + 1) * 128, 0:W], ew[:, :W])
                    # (c) attn_out, normalized into SBUF
                    for qi in range(4):
                        qt = 4 * Q + qi
                        po = psum.tile([128, 512], F32, tag="ps")
                        for kt in range(qt + 1):
                            nc.tensor.matmul(po[:, :256], ET[kt][:, qi * 128:(qi + 1) * 128],
                                             v_sb[kt][:], start=(kt == 0), stop=(kt == qt))
                        ao = aop.tile([128, 256], F32, tag="ao")
                        nc.vector.tensor_scalar_mul(ao[:], po[:, :256], RD[qi][:])
                        if DEBUG and Q == 0 and qi == 0:
                            nc.sync.dma_start(dbg_ao.ap()[h], ao[:])
                        AO[(h, qi)] = ao
                # (d/e) diff + RMSNorm + transpose + o_proj partial
                for qi in range(4):
                    qt = 4 * Q + qi
                    dift = sp.tile([128, 256], F32, tag="dift")
                    nc.vector.tensor_scalar_mul(dift[:], AO[(1, qi)][:], lam_sb[:])
                    dif = sp.tile([128, 256], F32, tag="dif")
                    nc.vector.tensor_sub(dif[:], AO[(0, qi)][:], dift[:])
                    sq = sp.tile([128, 256], F32, tag="sq")
                    ssum = dp.tile([128, 1], F32, tag="ss")
                    nc.scalar.activation(sq[:], dif[:], AF.Square, accum_out=ssum[:])
                    rms = dp.tile([128, 1], F32, tag="rms")
                    nc.scalar.activation(rms[:], ssum[:], AF.Sqrt, scale=1.0 / 256.0,
                                         bias=eps_sb[:])
                    rstd = dp.tile([128, 1], F32, tag="rstd")
                    nc.vector.reciprocal(rstd[:], rms[:])
                    act = sp.tile([128, 256], F32, tag="act")
                    nc.vector.tensor_scalar_mul(act[:], dif[:], rstd[:])
                    pt = psum.tile([128, 512], F32, tag="ps")
                    nc.tensor.transpose(pt[:, 0:128], act[:, 0:128], idm_sb[:])
                    nc.tensor.transpose(pt[:, 128:256], act[:, 128:256], idm_sb[:])
                    atT = sp.tile([128, 256], F32R, tag="atT")
                    nc.vector.tensor_copy(atT[:], pt[:, :256])
                    if DEBUG and Q == 0 and qi == 0:
                        nc.gpsimd.dma_start(dbg_act.ap(), act[:])
                        nc.gpsimd.dma_start(dbg_atT.ap(), atT[:])
                    osb = osp.tile([128, 2048], F32, tag="osb")
                    for nb in range(4):
                        pso = psum.tile([128, 512], F32, tag="ps")
                        for k2 in range(2):
                            nc.tensor.matmul(pso[:], atT[:, k2 * 128:(k2 + 1) * 128],
                                             wo_sb[k2][:, nb * 512:(nb + 1) * 512],
                                             start=(k2 == 0), stop=(k2 == 1))
                        dst = osb[:, nb * 512:(nb + 1) * 512]
                        if nb % 2 == 0:
                            nc.scalar.copy(dst, pso[:])
                        else:
                            nc.vector.tensor_copy(dst, pso[:])
                    nc.sync.dma_start(outp.ap()[qt * 128:(qt + 1) * 128, :], osb[:])
    nc.compile()
    return nc


def _get_nc():
    global _CACHED_NC
    if _CACHED_NC is None:
        _CACHED_NC = _build()
    return _CACHED_NC


def kernel(hidden_states, cos, sin, Wq, Wk, Wv, Wo,
           lambda_q1, lambda_k1, lambda_q2, lambda_k2):
    global LAST_EXEC_NS
    f = np.float32
    hs = np.asarray(hidden_states, f)[0]          # [S, HID]
    hsT = np.ascontiguousarray(hs.T)              # [HID, S]
    cosT = np.ascontiguousarray(np.asarray(cos, f).T)   # [D, S]
    sinT = np.ascontiguousarray(np.asarray(sin, f).T)
    sgn = np.concatenate([-np.ones(64, f), np.ones(64, f)])[:, None]
    cosq = cosT * f(INV_SQRT_D)
    sinq = sinT * sgn * f(INV_SQRT_D)
    cosk = cosT
    sink = sinT * sgn

    lam1 = np.exp(np.sum(np.asarray(lambda_q1, f) * np.asarray(lambda_k1, f),
                         dtype=np.float64))
    lam2 = np.exp(np.sum(np.asarray(lambda_q2, f) * np.asarray(lambda_k2, f),
                         dtype=np.float64))
    lam = f(lam1 - lam2 + LAMBDA_INIT)
    lamv = np.full((128, 1), lam, f)

    r = np.arange(128)
    maskq = np.where(r[None, :] <= r[:, None], f(0.0), f(-1e9)).astype(f)  # [sq, sk]
    maskt = np.ascontiguousarray(maskq.T)
    idm = np.eye(128, dtype=f)

    Wq_, Wk_, Wv_, Wo_ = (np.asarray(x, f) for x in (Wq, Wk, Wv, Wo))
    Wo_s = Wo_ * f(1.0 - LAMBDA_INIT)

    in_maps = []
    for i in range(NCORES):
        a, b = i // 2, 4 + i // 2
        wq_i = np.ascontiguousarray(
            np.concatenate([Wq_[:, i * D:(i + 1) * D],
                            Wq_[:, (i + 8) * D:(i + 9) * D]], axis=1))
        wk_i = np.ascontiguousarray(
            np.concatenate([Wk_[:, a * D:(a + 1) * D],
                            Wk_[:, b * D:(b + 1) * D]], axis=1))
        wv_i = np.ascontiguousarray(
            np.concatenate([Wv_[:, a * D:(a + 1) * D],
                            Wv_[:, b * D:(b + 1) * D]], axis=1))
        wo_i = np.ascontiguousarray(Wo_s[i * 256:(i + 1) * 256, :])
        in_maps.append(dict(hsT=hsT, wq=wq_i, wk=wk_i, wv=wv_i, wo=wo_i,
                            cosq=cosq, sinq=sinq, cosk=cosk, sink=sink,
                            lamv=lamv, maskq=maskq, maskt=maskt, idm=idm))

    nc = _get_nc()
    kw = {}
    if TRACE:
        import concourse.bass_utils as _bu
        _bu.upload_artifacts = lambda tmpdir: tmpdir  # no artifact store here
        os.makedirs("/root/problem/traces", exist_ok=True)
        tdir = tempfile.mkdtemp(prefix="trace_", dir="/root/problem/traces")
        kw = dict(trace=True, trace_cores=list(range(NCORES)), tmpdir=tdir)
    res = run_bass_kernel_spmd(nc, in_maps, core_ids=list(range(NCORES)), **kw)
    LAST_EXEC_NS = res.exec_time_ns
    global LAST_RESULTS
    LAST_RESULTS = res.results

    attn_full = np.zeros((1, H, S, S), f)
    out = np.zeros((S, HID), f)
    for i in range(NCORES):
        attn_full[0, i] = res.results[i]["attnw"][0]
        attn_full[0, i + 8] = res.results[i]["attnw"][1]
        out += res.results[i]["outp"]
    return out.reshape(B, S, HID), attn_full


# revision 18
# speedup vs baseline: 1.0950x; 1.0407x over previous
"""DiffLlama attention (B=1, S=2048, HID=2048, H=16, KVH=8, D=128) on 8 TRN2 cores.

Sharding: tensor-parallel over the 8 "effective" (differential) heads.
Core i owns query heads (i, i+8), kv heads (i//2, 4+i//2), and the matching
256-column slice of the v_cat / output projection. o_proj is row-sharded;
partial products are summed on the host. attn_weights upper triangle is never
written on device (PJRT zero-fills outputs), matching softmax's exact zeros.

All matmuls run in float32r (full-rate PE mode, ~2^-12 effective rounding).
"""
import math
import os
import tempfile
import numpy as np
from contextlib import ExitStack

import concourse.bass as bass
import concourse.tile as tile
from concourse import bacc, mybir
from concourse.bass_utils import run_bass_kernel_spmd

B, S, HID = 1, 2048, 2048
H, KVH, D = 16, 8, 128
NCORES = 8
LAYER_IDX = 1
LAMBDA_INIT = 0.8 - 0.6 * float(np.exp(-0.3 * LAYER_IDX))
EPS = 1e-6
INV_SQRT_D = 1.0 / math.sqrt(D)

F32 = mybir.dt.float32
F32R = mybir.dt.float32r
BF16 = mybir.dt.bfloat16
ATTN_BF16 = True   # bf16 for the S^T/ET/v/attn_out island (attnw path stays f32r)
AF = mybir.ActivationFunctionType

NQT = S // 128   # 16 query/key row tiles
NCH = S // 512   # 4 seq chunks

TRACE = False          # set by test.py to profile
DEBUG = False          # extra intermediate outputs for debugging
LAST_EXEC_NS = None
LAST_RESULTS = None
_CACHED_NC = None


def _build():
    nc = bacc.Bacc("TRN2", target_bir_lowering=False, debug=False)

    hsT = nc.dram_tensor("hsT", [HID, S], F32R, kind="ExternalInput")
    wq = nc.dram_tensor("wq", [HID, 256], F32R, kind="ExternalInput")
    wk = nc.dram_tensor("wk", [HID, 256], F32R, kind="ExternalInput")
    wv = nc.dram_tensor("wv", [HID, 256], F32R, kind="ExternalInput")
    wo = nc.dram_tensor("wo", [256, HID], F32R, kind="ExternalInput")
    cosq = nc.dram_tensor("cosq", [D, S], F32, kind="ExternalInput")
    sinq = nc.dram_tensor("sinq", [D, S], F32, kind="ExternalInput")
    cosk = nc.dram_tensor("cosk", [D, S], F32, kind="ExternalInput")
    sink = nc.dram_tensor("sink", [D, S], F32, kind="ExternalInput")
    lamv = nc.dram_tensor("lamv", [128, 1], F32, kind="ExternalInput")
    maskq = nc.dram_tensor("maskq", [128, 128], F32, kind="ExternalInput")
    maskt = nc.dram_tensor("maskt", [128, 128], F32, kind="ExternalInput")
    idm = nc.dram_tensor("idm", [128, 128], F32, kind="ExternalInput")

    attnw = nc.dram_tensor("attnw", [2, S, S], F32, kind="ExternalOutput")
    outp = nc.dram_tensor("outp", [S, HID], F32, kind="ExternalOutput")
    if DEBUG:
        dbg_et = nc.dram_tensor("dbg_et", [128, 512], F32R, kind="ExternalOutput")
        dbg_ao = nc.dram_tensor("dbg_ao", [2, 128, 256], F32, kind="ExternalOutput")
        dbg_act = nc.dram_tensor("dbg_act", [128, 256], F32, kind="ExternalOutput")
        dbg_atT = nc.dram_tensor("dbg_atT", [128, 256], F32R, kind="ExternalOutput")
        dbg_v = nc.dram_tensor("dbg_v", [128, 256], F32, kind="ExternalOutput")

    with tile.TileContext(nc) as tc, ExitStack() as ctx:
        # ---- persistent pools ----
        psum = ctx.enter_context(tc.tile_pool(name="psum", bufs=4, space="PSUM"))
        qk = ctx.enter_context(tc.tile_pool(name="qk", bufs=1))
        vp = ctx.enter_context(tc.tile_pool(name="vp", bufs=16))
        wop = ctx.enter_context(tc.tile_pool(name="wop", bufs=2))
        consts = ctx.enter_context(tc.tile_pool(name="consts", bufs=1))

        qT = [qk.tile([128, S], F32R, tag=f"qT{h}", name=f"qT{h}") for h in range(2)]
        kT = [qk.tile([128, S], F32R, tag=f"kT{h}", name=f"kT{h}") for h in range(2)]
        DT_A = BF16 if ATTN_BF16 else F32R
        v_sb = [vp.tile([128, 256], DT_A, tag="v", name=f"v{j}") for j in range(NQT)]
        wo_sb = [wop.tile([128, S], F32R, tag="wo", name=f"wo{j}") for j in range(2)]

        maskq_sb = consts.tile([128, 128], F32, tag="mq")
        maskt_sb = consts.tile([128, 128], F32, tag="mt")
        idm_sb = consts.tile([128, 128], F32, tag="idm")
        lam_sb = consts.tile([128, 1], F32, tag="lam")
        eps_sb = consts.tile([128, 1], F32, tag="eps")
        nc.vector.memset(eps_sb[:], EPS)
        nc.gpsimd.dma_start(maskq_sb[:], maskq.ap())
        nc.gpsimd.dma_start(maskt_sb[:], maskt.ap())
        nc.gpsimd.dma_start(idm_sb[:], idm.ap())
        nc.gpsimd.dma_start(lam_sb[:], lamv.ap())
        for t2 in range(2):
            nc.gpsimd.dma_start(wo_sb[t2][:], wo.ap()[t2 * 128:(t2 + 1) * 128, :])

        # ================= stage 1: projections + RoPE =================
        with ExitStack() as s1:
            wts = s1.enter_context(tc.tile_pool(name="wts", bufs=1))
            hsp = s1.enter_context(tc.tile_pool(name="hsp", bufs=6))
            trig = s1.enter_context(tc.tile_pool(name="trig", bufs=1))
            rope = s1.enter_context(tc.tile_pool(name="rope", bufs=3))

            # resident weights: [128, 16, 256] (hid-tile-major)
            wq_sb = wts.tile([128, NQT, 256], F32R, tag="wq")
            wk_sb = wts.tile([128, NQT, 256], F32R, tag="wk")
            wv_sb = wts.tile([128, NQT, 256], F32R, tag="wv")
            for kt in range(NQT):
                for dsrc, dst in ((wq, wq_sb), (wk, wk_sb), (wv, wv_sb)):
                    nc.scalar.dma_start(dst[:, kt, :],
                                        dsrc.ap()[kt * 128:(kt + 1) * 128, :])

            cs = {}
            for name, dsrc in (("cosq", cosq), ("sinq", sinq), ("cosk", cosk), ("sink", sink)):
                t = trig.tile([128, S], F32, tag=name)
                for c4 in range(4):
                    nc.gpsimd.dma_start(t[:, c4 * 512:(c4 + 1) * 512],
                                        dsrc.ap()[:, c4 * 512:(c4 + 1) * 512])
                cs[name] = t

            for n in range(NCH):
                sl = slice(n * 512, (n + 1) * 512)
                psqk0 = psum.tile([128, 1024], F32, tag="ps2", name=f"psqk0_{n}", bufs=2)
                psqk1 = psum.tile([128, 1024], F32, tag="ps2", name=f"psqk1_{n}", bufs=2)
                ps_q = [psqk0[:, 0:512], psqk0[:, 512:1024]]
                ps_k = [psqk1[:, 0:512], psqk1[:, 512:1024]]
                # one bank per accumulation group (start= clears whole-bank bits)
                ps_v = [psum.tile([128, 512], F32, tag="ps", name=f"psv{j}") for j in range(4)]
                for kt in range(NQT):
                    hst = hsp.tile([128, 512], F32R, tag="hs")
                    dma_eng = nc.sync if kt % 2 == 0 else nc.scalar
                    dma_eng.dma_start(hst[:], hsT.ap()[kt * 128:(kt + 1) * 128, sl])
                    st, sp = kt == 0, kt == NQT - 1
                    for h in range(2):
                        nc.tensor.matmul(ps_q[h][:], wq_sb[:, kt, h * 128:(h + 1) * 128],
                                         hst[:], start=st, stop=sp)
                        nc.tensor.matmul(ps_k[h][:], wk_sb[:, kt, h * 128:(h + 1) * 128],
                                         hst[:], start=st, stop=sp)
                    for m in range(4):
                        nc.tensor.matmul(ps_v[m][:, :256],
                                         hst[:, m * 128:(m + 1) * 128], wv_sb[:, kt, :],
                                         start=st, stop=sp)
                # v copies (rounding to f32r)
                for m in range(4):
                    nc.vector.tensor_copy(v_sb[n * 4 + m][:], ps_v[m][:, :256])
                if DEBUG and n == 0:
                    pass  # dbg_v disabled
                # RoPE on q (scaled tables) and k
                for (ps_x, dstT, cname, sname) in (
                    (ps_q[0], qT[0], "cosq", "sinq"), (ps_q[1], qT[1], "cosq", "sinq"),
                    (ps_k[0], kT[0], "cosk", "sink"), (ps_k[1], kT[1], "cosk", "sink"),
                ):
                    xs = rope.tile([128, 512], F32, tag="xs")
                    nc.vector.tensor_copy(xs[:], ps_x[:])
                    xsh = rope.tile([128, 512], F32, tag="xsh")
                    nc.gpsimd.dma_start(xsh[0:64, :], xs[64:128, :])
                    nc.gpsimd.dma_start(xsh[64:128, :], xs[0:64, :])
                    t1 = rope.tile([128, 512], F32, tag="t1")
                    nc.vector.tensor_mul(t1[:], xs[:], cs[cname][:, sl])
                    t2 = rope.tile([128, 512], F32, tag="t2")
                    nc.vector.tensor_mul(t2[:], xsh[:], cs[sname][:, sl])
                    nc.vector.tensor_add(dstT[:, sl], t1[:], t2[:])

        # ================= stage 2: attention =================
        with ExitStack() as s2:
            etp = s2.enter_context(tc.tile_pool(name="etp", bufs=16))
            ewp = s2.enter_context(tc.tile_pool(name="ewp", bufs=3))
            aop = s2.enter_context(tc.tile_pool(name="aop", bufs=8))
            dp = s2.enter_context(tc.tile_pool(name="dp", bufs=24))
            sp = s2.enter_context(tc.tile_pool(name="sp", bufs=3))
            osp = s2.enter_context(tc.tile_pool(name="osp", bufs=3))

            if ATTN_BF16:
                qTb = [qk.tile([128, S], BF16, tag=f"qTb{h}", name=f"qTb{h}")
                       for h in range(2)]
                kTb = [qk.tile([128, S], BF16, tag=f"kTb{h}", name=f"kTb{h}")
                       for h in range(2)]
                for h in range(2):
                    nc.vector.tensor_copy(qTb[h][:], qT[h][:])
                    nc.vector.tensor_copy(kTb[h][:], kT[h][:])
            else:
                qTb, kTb = qT, kT

            for Q in range(NCH):
                kmax = 4 * (Q + 1)
                qsl = slice(Q * 512, (Q + 1) * 512)
                AO = {}
                for h in range(2):
                    # (a) transposed score strips -> exp -> ET[kt]
                    ET = []
                    for kt in range(kmax):
                        ps = psum.tile([128, 512], F32, tag="ps", name=f"pst{kt}")
                        nc.tensor.matmul(ps[:], kTb[h][:, kt * 128:(kt + 1) * 128],
                                         qTb[h][:, qsl], start=True, stop=True)
                        if kt >= 4 * Q:
                            sub = slice((kt - 4 * Q) * 128, (kt - 4 * Q) * 128 + 128)
                            nc.vector.tensor_add(ps[:, sub], ps[:, sub], maskt_sb[:])
                        et = etp.tile([128, 512], DT_A, tag="et")
                        nc.scalar.activation(et[:], ps[:], AF.Exp)
                        if DEBUG and Q == 0 and h == 0 and kt == 0:
                            nc.sync.dma_start(dbg_et.ap(), et[:])
                        ET.append(et)
                    # (b) natural scores -> exp(+rowsum) -> normalize -> attnw out
                    RD = []
                    for qi in range(4):
                        qt = 4 * Q + qi
                        W = (qt + 1) * 128
                        nchq = (W + 511) // 512
                        halves = []
                        for hv in range((W + 1023) // 1024):
                            wh = min(1024, W - hv * 1024)
                            p2 = psum.tile([128, 1024], F32, tag="ps2",
                                           name=f"p2n{qt}{h}{hv}", bufs=2)
                            for c2 in range((wh + 511) // 512):
                                w = min(512, wh - c2 * 512)
                                co = hv * 1024 + c2 * 512
                                nc.tensor.matmul(p2[:, c2 * 512:c2 * 512 + w],
                                                 qT[h][:, qt * 128:(qt + 1) * 128],
                                                 kT[h][:, co:co + w],
                                                 start=True, stop=True)
                            halves.append((p2, wh))
                        lastp, lastw = halves[-1]
                        nc.vector.tensor_add(lastp[:, lastw - 128:lastw],
                                             lastp[:, lastw - 128:lastw], maskq_sb[:])
                        ew = ewp.tile([128, 2048], F32, tag="ew")
                        dens = []
                        for hv, (p2, wh) in enumerate(halves):
                            dpt = dp.tile([128, 1], F32, tag="dp")
                            nc.scalar.activation(ew[:, hv * 1024:hv * 1024 + wh],
                                                 p2[:, :wh], AF.Exp, accum_out=dpt[:])
                            dens.append(dpt)
                        den = dens[0]
                        if len(dens) > 1:
                            den2 = dp.tile([128, 1], F32, tag="dp")
                            nc.vector.tensor_add(den2[:], dens[0][:], dens[1][:])
                            den = den2
                        rd = dp.tile([128, 1], F32, tag="rd")
                        nc.vector.reciprocal(rd[:], den[:])
                        RD.append(rd)
                        nc.vector.tensor_scalar_mul(ew[:, :W], ew[:, :W], rd[:])
                        nc.sync.dma_start(
                            attnw.ap()[h, qt * 128:(qt + 1) * 128, 0:W], ew[:, :W])
                    # (c) attn_out, normalized into SBUF
                    for qi in range(4):
                        qt = 4 * Q + qi
                        po = psum.tile([128, 512], F32, tag="ps")
                        for kt in range(qt + 1):
                            nc.tensor.matmul(po[:, :256], ET[kt][:, qi * 128:(qi + 1) * 128],
                                             v_sb[kt][:], start=(kt == 0), stop=(kt == qt))
                        ao = aop.tile([128, 256], F32, tag="ao")
                        nc.vector.tensor_scalar_mul(ao[:], po[:, :256], RD[qi][:])
                        if DEBUG and Q == 0 and qi == 0:
                            nc.sync.dma_start(dbg_ao.ap()[h], ao[:])
                        AO[(h, qi)] = ao
                # (d/e) diff + RMSNorm + transpose + o_proj partial
                for qi in range(4):
                    qt = 4 * Q + qi
                    dift = sp.tile([128, 256], F32, tag="dift")
                    nc.vector.tensor_scalar_mul(dift[:], AO[(1, qi)][:], lam_sb[:])
                    dif = sp.tile([128, 256], F32, tag="dif")
                    nc.vector.tensor_sub(dif[:], AO[(0, qi)][:], dift[:])
                    sq = sp.tile([128, 256], F32, tag="sq")
                    ssum = dp.tile([128, 1], F32, tag="ss")
                    nc.scalar.activation(sq[:], dif[:], AF.Square, accum_out=ssum[:])
                    rms = dp.tile([128, 1], F32, tag="rms")
                    nc.scalar.activation(rms[:], ssum[:], AF.Sqrt, scale=1.0 / 256.0,
                                         bias=eps_sb[:])
                    rstd = dp.tile([128, 1], F32, tag="rstd")
                    nc.vector.reciprocal(rstd[:], rms[:])
                    act = sp.tile([128, 256], F32, tag="act")
                    nc.vector.tensor_scalar_mul(act[:], dif[:], rstd[:])
                    pt = psum.tile([128, 512], F32, tag="ps")
                    nc.tensor.transpose(pt[:, 0:128], act[:, 0:128], idm_sb[:])
                    nc.tensor.transpose(pt[:, 128:256], act[:, 128:256], idm_sb[:])
                    atT = sp.tile([128, 256], F32R, tag="atT")
                    nc.vector.tensor_copy(atT[:], pt[:, :256])
                    if DEBUG and Q == 0 and qi == 0:
                        nc.gpsimd.dma_start(dbg_act.ap(), act[:])
                        nc.gpsimd.dma_start(dbg_atT.ap(), atT[:])
                    osb = osp.tile([128, 2048], F32, tag="osb")
                    for nb in range(4):
                        pso = psum.tile([128, 512], F32, tag="ps")
                        for k2 in range(2):
                            nc.tensor.matmul(pso[:], atT[:, k2 * 128:(k2 + 1) * 128],
                                             wo_sb[k2][:, nb * 512:(nb + 1) * 512],
                                             start=(k2 == 0), stop=(k2 == 1))
                        dst = osb[:, nb * 512:(nb + 1) * 512]
                        if nb % 2 == 0:
                            nc.scalar.copy(dst, pso[:])
                        else:
                            nc.vector.tensor_copy(dst, pso[:])
                    nc.sync.dma_start(outp.ap()[qt * 128:(qt + 1) * 128, :], osb[:])
    nc.compile()
    return nc


def _get_nc():
    global _CACHED_NC
    if _CACHED_NC is None:
        _CACHED_NC = _build()
    return _CACHED_NC


def kernel(hidden_states, cos, sin, Wq, Wk, Wv, Wo,
           lambda_q1, lambda_k1, lambda_q2, lambda_k2):
    global LAST_EXEC_NS
    f = np.float32
    hs = np.asarray(hidden_states, f)[0]          # [S, HID]
    hsT = np.ascontiguousarray(hs.T)              # [HID, S]
    cosT = np.ascontiguousarray(np.asarray(cos, f).T)   # [D, S]
    sinT = np.ascontiguousarray(np.asarray(sin, f).T)
    sgn = np.concatenate([-np.ones(64, f), np.ones(64, f)])[:, None]
    cosq = cosT * f(INV_SQRT_D)
    sinq = sinT * sgn * f(INV_SQRT_D)
    cosk = cosT
    sink = sinT * sgn

    lam1 = np.exp(np.sum(np.asarray(lambda_q1, f) * np.asarray(lambda_k1, f),
                         dtype=np.float64))
    lam2 = np.exp(np.sum(np.asarray(lambda_q2, f) * np.asarray(lambda_k2, f),
                         dtype=np.float64))
    lam = f(lam1 - lam2 + LAMBDA_INIT)
    lamv = np.full((128, 1), lam, f)

    r = np.arange(128)
    maskq = np.where(r[None, :] <= r[:, None], f(0.0), f(-1e9)).astype(f)  # [sq, sk]
    maskt = np.ascontiguousarray(maskq.T)
    idm = np.eye(128, dtype=f)

    Wq_, Wk_, Wv_, Wo_ = (np.asarray(x, f) for x in (Wq, Wk, Wv, Wo))
    Wo_s = Wo_ * f(1.0 - LAMBDA_INIT)

    in_maps = []
    for i in range(NCORES):
        a, b = i // 2, 4 + i // 2
        wq_i = np.ascontiguousarray(
            np.concatenate([Wq_[:, i * D:(i + 1) * D],
                            Wq_[:, (i + 8) * D:(i + 9) * D]], axis=1))
        wk_i = np.ascontiguousarray(
            np.concatenate([Wk_[:, a * D:(a + 1) * D],
                            Wk_[:, b * D:(b + 1) * D]], axis=1))
        wv_i = np.ascontiguousarray(
            np.concatenate([Wv_[:, a * D:(a + 1) * D],
                            Wv_[:, b * D:(b + 1) * D]], axis=1))
        wo_i = np.ascontiguousarray(Wo_s[i * 256:(i + 1) * 256, :])
        in_maps.append(dict(hsT=hsT, wq=wq_i, wk=wk_i, wv=wv_i, wo=wo_i,
                            cosq=cosq, sinq=sinq, cosk=cosk, sink=sink,
                            lamv=lamv, maskq=maskq, maskt=maskt, idm=idm))

    nc = _get_nc()
    kw = {}
    if TRACE:
        import concourse.bass_utils as _bu
        _bu.upload_artifacts = lambda tmpdir: tmpdir  # no artifact store here
        os.makedirs("/root/problem/traces", exist_ok=True)
        tdir = tempfile.mkdtemp(prefix="trace_", dir="/root/problem/traces")
        kw = dict(trace=True, trace_cores=list(range(NCORES)), tmpdir=tdir)
    res = run_bass_kernel_spmd(nc, in_maps, core_ids=list(range(NCORES)), **kw)
    LAST_EXEC_NS = res.exec_time_ns
    global LAST_RESULTS
    LAST_RESULTS = res.results

    attn_full = np.zeros((1, H, S, S), f)
    out = np.zeros((S, HID), f)
    for i in range(NCORES):
        attn_full[0, i] = res.results[i]["attnw"][0]
        attn_full[0, i + 8] = res.results[i]["attnw"][1]
        out += res.results[i]["outp"]
    return out.reshape(B, S, HID), attn_full
